# revision 50
# baseline (speedup 1.0000x reference)
"""MOLELinear (mixture-of-linear-experts) Trainium2 kernel.

Math (per group g): out_g = x_g @ (sum_e c[g,e] W_e + W_sh).T + (sum_e c[g,e] b_e + b_sh)

Sharding: data-parallel over the 32 groups -> 4 groups (8192 tokens) per core,
expert weights replicated. Host does layout-only prep (transpose / stacking /
fp16 rounding); all arithmetic of the reference runs on device.

Device plan per core (all fp16 data path, fp32 accumulation in PSUM):
  - Weights arrive in two o-halves (wallA = out-feature tiles 0-1, wallB =
    tiles 2-3), each half expert-contiguous, so the first GEMM phases start
    after only half the weight bytes have landed.
  - Group-0 weight mix on PE during the weight-DMA window via scaled-identity
    diagonal matmuls (fp32 PSUM accumulation), per o-half.
  - Groups 1-3 weight mix on DVE as per-half FMA chains:
    tensor_scalar (4x mode) + tensor_tensor (2x mode), FD=1024.
  - Mixed biases transposed on PE: mbT2[o', ot*4+g] = sum_j ball[j,o]cx[j,g].
  - Main GEMM: stationary = mixed-weight subtile [128k,128o], moving = xT
    slice [128k,512t]; psum [128 o', 1024 t] (2 banks) over 4 k-tiles.
    Phase order interleaves groups by weight half so DVE mixing keeps up.
  - Drain on ScalarE via ACTIVATE(Identity, bias=mbT2 column) -> fp16 SBUF;
    output stores on the second HWDGE ring (nc.scalar).
"""
import numpy as np

import concourse.bacc as bacc
import concourse.mybir as mybir
from concourse.alu_op_type import AluOpType
from concourse.tile import TileContext
from concourse.bass_utils import run_bass_kernel_spmd

N_CORES = 8
IN_F = 512
OUT_F = 512
N_EXPERTS = 8
N_GROUPS = 32
TOK_PER_GROUP = 2048
G_PER_CORE = N_GROUPS // N_CORES           # 4
TOK_PER_CORE = G_PER_CORE * TOK_PER_GROUP  # 8192
KT = IN_F // 128                           # 4 k-tiles
OT = OUT_F // 128                          # 4 out-feature tiles
NW = N_EXPERTS + 1                         # 9: shared weight first, then experts
HALF = KT * OUT_F // 2                     # 1024 columns per o-half
F32 = mybir.dt.float32
F32R = mybir.dt.float32r
F16 = mybir.dt.float16
AF = mybir.ActivationFunctionType

_CACHE = {}


def _build():
    nc = bacc.Bacc(trn_type="TRN2")
    xT = nc.dram_tensor("xT", (IN_F, TOK_PER_CORE), F16, kind="ExternalInput")
    wallA0 = nc.dram_tensor("wallA0", (128, NW, 512), F16, kind="ExternalInput")
    wallA1 = nc.dram_tensor("wallA1", (128, NW, 512), F16, kind="ExternalInput")
    wallB0 = nc.dram_tensor("wallB0", (128, NW, 512), F16, kind="ExternalInput")
    wallB1 = nc.dram_tensor("wallB1", (128, NW, 512), F16, kind="ExternalInput")
    cdiag = nc.dram_tensor("cdiag", (128, NW, 128), F16, kind="ExternalInput")
    cb = nc.dram_tensor("cb", (128, G_PER_CORE * N_EXPERTS), F32, kind="ExternalInput")
    cx = nc.dram_tensor("cx", (NW, G_PER_CORE), F32R, kind="ExternalInput")
    ball = nc.dram_tensor("ball", (NW, OUT_F), F32R, kind="ExternalInput")
    outT = nc.dram_tensor("outT", (OUT_F, TOK_PER_CORE), F16, kind="ExternalOutput")

    with TileContext(nc) as tc:
        with (
            tc.tile_pool(name="smallp", bufs=1) as smallp,
            tc.tile_pool(name="wallp", bufs=1) as wallp,
            tc.tile_pool(name="wmp", bufs=1) as wmp,
            tc.tile_pool(name="xp", bufs=1) as xp,
            tc.tile_pool(name="ocp", bufs=4) as ocp,
        ):
            # ---- front DMAs: diag matrices + first weight quarter first ----
            cdt = smallp.tile([128, NW * 128], F16, tag="cdt")
            nc.sync.dma_start(cdt[:].rearrange("p (e m) -> p e m", e=NW), cdiag[:])

            # weight halves arrive as ot-quarters so the group-0 diag mix
            # (range-dep on each quarter) starts as early as possible;
            # group-0 x interleaved between them
            wallsA = wallp.tile([128, NW * HALF], F16, tag="wallsA")
            wallsA_r = wallsA[:].rearrange("p (e c) -> p e c", e=NW)
            nc.sync.dma_start(wallsA_r[:, :, 0:512], wallA0[:])
            cxt = smallp.tile([NW, G_PER_CORE], F32R, tag="cx")
            nc.sync.dma_start(cxt[:], cx[:])
            cbt = smallp.tile([128, G_PER_CORE * N_EXPERTS], F32, tag="cb")
            nc.sync.dma_start(cbt[:], cb[:])
            ballt = smallp.tile([NW, OUT_F], F32R, tag="ball")
            nc.sync.dma_start(ballt[:], ball[:])
            nc.sync.dma_start(wallsA_r[:, :, 512:1024], wallA1[:])

            wallsB = wallp.tile([128, NW * HALF], F16, tag="wallsB")
            wallsB_r = wallsB[:].rearrange("p (e c) -> p e c", e=NW)
            nc.sync.dma_start(wallsB_r[:, :, 0:512], wallB0[:])
            nc.sync.dma_start(wallsB_r[:, :, 512:1024], wallB1[:])

            xg0t = [
                xp.tile([128, TOK_PER_GROUP], F16, tag=f"x0k{kt}", name=f"x0k{kt}")
                for kt in range(KT)
            ]
            for kt in range(KT):
                nc.sync.dma_start(
                    xg0t[kt][:], xT[kt * 128 : (kt + 1) * 128, 0:TOK_PER_GROUP]
                )

            xg = [None]
            for g in range(1, G_PER_CORE):
                t = xp.tile([128, KT * TOK_PER_GROUP], F16, tag=f"x{g}", name=f"x{g}")
                nc.sync.dma_start(
                    t[:].rearrange("p (kt t) -> p kt t", kt=KT),
                    xT[:, g * TOK_PER_GROUP : (g + 1) * TOK_PER_GROUP].rearrange(
                        "(kt p) t -> p kt t", p=128
                    ),
                )
                xg.append(t)

            walls = [wallsA, wallsB]

            def wsl(h, j, c0, c1):
                return walls[h][:, j * HALF + c0 : j * HALF + c1]

            def xslice(g, kt, tci):
                if g == 0:
                    return xg0t[kt][:, tci * 512 : (tci + 1) * 512]
                return xg[g][
                    :,
                    kt * TOK_PER_GROUP + tci * 512 : kt * TOK_PER_GROUP + (tci + 1) * 512,
                ]

            # wm layout per group: column = ot*512 + kt*128 + o'  (o-major)
            wm = [
                wmp.tile([128, KT * OUT_F], F16, tag=f"wm{g}", name=f"wm{g}")
                for g in range(G_PER_CORE)
            ]

            with (
                tc.tile_pool(name="psd", bufs=1, space="PSUM") as psd,
                tc.tile_pool(name="ps", bufs=3, space="PSUM") as ps,
            ):
                # ---- mixed biases (rides the psd slot, freed early) ----
                pb = psd.tile([128, HALF], F32, tag="psd", name="pb")
                for ot in range(OT):
                    nc.tensor.matmul(
                        pb[:, ot * G_PER_CORE : (ot + 1) * G_PER_CORE],
                        ballt[:, ot * 128 : (ot + 1) * 128],
                        cxt[:],
                        start=True,
                        stop=True,
                    )
                mbT2 = smallp.tile([128, OT * G_PER_CORE], F32, tag="mbT2")
                nc.scalar.copy(mbT2[:], pb[:, 0 : OT * G_PER_CORE])

                # ---- group-0 weight mix on PE via scaled-identity diagonal
                # matmuls, one o-half at a time (runs during the DMA ramp) ----
                def diag_mix(h):
                    pm = psd.tile([128, HALF], F32, tag="psd", name=f"pm{h}")
                    for otl in range(2):
                        for j in range(NW):
                            nc.tensor.matmul(
                                pm[:, otl * 512 : (otl + 1) * 512],
                                cdt[:, j * 128 : (j + 1) * 128],
                                wsl(h, j, otl * 512, (otl + 1) * 512),
                                start=(j == 0),
                                stop=(j == NW - 1),
                            )
                        nc.scalar.copy(
                            wm[0][
                                :, h * HALF + otl * 512 : h * HALF + (otl + 1) * 512
                            ],
                            pm[:, otl * 512 : (otl + 1) * 512],
                        )

                diag_mix(0)
                diag_mix(1)  # both halves now complete within the DMA ramp

                # ---- groups 1-3 weight mix on DVE, per half ----
                def mix_chain(g, h):
                    for e in range(N_EXPERTS):
                        tmp = wmp.tile([128, HALF], F16, tag="tmp", name="tmp", bufs=2)
                        nc.vector.tensor_scalar(
                            tmp[:],
                            wsl(h, e + 1, 0, HALF),
                            cbt[:, g * N_EXPERTS + e : g * N_EXPERTS + e + 1],
                            None,
                            AluOpType.mult,
                        )
                        nc.vector.tensor_tensor(
                            wm[g][:, h * HALF : (h + 1) * HALF],
                            tmp[:],
                            wsl(h, 0, 0, HALF)
                            if e == 0
                            else wm[g][:, h * HALF : (h + 1) * HALF],
                            AluOpType.add,
                        )

                for g, h in ((1, 0), (2, 0), (1, 1), (2, 1), (3, 0), (3, 1)):
                    mix_chain(g, h)

                # ---- main GEMM: phases interleaved by weight half; group-0
                # second-half diag mix inserted after the first two phases ----
                phase_order = [
                    (0, 0), (0, 1), (0, 2), (0, 3),
                    (1, 0), (1, 1), (2, 0), (2, 1),
                    (1, 2), (1, 3), (2, 2), (2, 3),
                    (3, 0), (3, 1), (3, 2), (3, 3),
                ]
                n_phase = len(phase_order)
                for pi, (g, ot) in enumerate(phase_order):
                    oc = ocp.tile([128, TOK_PER_GROUP], F16, tag="oc", name="oc")
                    bias_ap = mbT2[:, ot * G_PER_CORE + g : ot * G_PER_CORE + g + 1]
                    last = pi == n_phase - 1
                    for th in range(2):
                        pt = ps.tile([128, 1024], F32, tag="ps", name="pt")
                        for kt in range(KT):
                            lhsT = wm[g][
                                :, ot * 512 + kt * 128 : ot * 512 + (kt + 1) * 128
                            ]
                            for tci in range(2):
                                nc.tensor.matmul(
                                    pt[:, tci * 512 : (tci + 1) * 512],
                                    lhsT,
                                    xslice(g, kt, th * 2 + tci),
                                    start=(kt == 0),
                                    stop=(kt == KT - 1),
                                )
                        if last and th == 1:
                            # final drain on DVE, parallel to ScalarE's th0 drain
                            nc.vector.tensor_scalar(
                                oc[:, th * 1024 : (th + 1) * 1024],
                                pt[:],
                                bias_ap,
                                None,
                                AluOpType.add,
                            )
                        else:
                            nc.scalar.activation(
                                oc[:, th * 1024 : (th + 1) * 1024],
                                pt[:],
                                AF.Identity,
                                bias=bias_ap,
                                scale=1.0,
                            )
                        if last:
                            nc.scalar.dma_start(
                                outT[
                                    ot * 128 : (ot + 1) * 128,
                                    g * TOK_PER_GROUP
                                    + th * 1024 : g * TOK_PER_GROUP
                                    + (th + 1) * 1024,
                                ],
                                oc[:, th * 1024 : (th + 1) * 1024],
                            )
                    if not last:
                        nc.scalar.dma_start(
                            outT[
                                ot * 128 : (ot + 1) * 128,
                                g * TOK_PER_GROUP : (g + 1) * TOK_PER_GROUP,
                            ],
                            oc[:],
                        )
    nc.finalize()
    return nc


def kernel(x, coefficients, weight_experts, bias_experts, weight_shared, bias_shared, sizes):
    x = np.asarray(x)
    coefficients = np.asarray(coefficients, dtype=np.float32)
    weight_experts = np.asarray(weight_experts, dtype=np.float32)
    bias_experts = np.asarray(bias_experts, dtype=np.float32)
    weight_shared = np.asarray(weight_shared, dtype=np.float32)
    bias_shared = np.asarray(bias_shared, dtype=np.float32)

    if "nc" not in _CACHE:
        _CACHE["nc"] = _build()
    nc = _CACHE["nc"]

    # ---- host-side layout prep ----
    x16 = x.astype(np.float16)
    # per expert j: X[p, ot, kt, o'] = W_j^T[kt*128+p, ot*128+o']
    # wallA = out-feature tiles 0-1, wallB = tiles 2-3 (each [128, 9, 1024])
    wallA0_np = np.empty((128, NW, 512), np.float16)
    wallA1_np = np.empty((128, NW, 512), np.float16)
    wallB0_np = np.empty((128, NW, 512), np.float16)
    wallB1_np = np.empty((128, NW, 512), np.float16)
    for j in range(NW):
        W = weight_shared if j == 0 else weight_experts[j - 1]
        X = (
            W.T.reshape(KT, 128, OT, 128)
            .transpose(1, 2, 0, 3)
            .astype(np.float16)
        )  # [p, ot, kt, o']
        wallA0_np[:, j, :] = X[:, 0].reshape(128, 512)
        wallA1_np[:, j, :] = X[:, 1].reshape(128, 512)
        wallB0_np[:, j, :] = X[:, 2].reshape(128, 512)
        wallB1_np[:, j, :] = X[:, 3].reshape(128, 512)
    ball_np = np.empty((NW, OUT_F), np.float32)
    ball_np[0] = bias_shared
    ball_np[1:] = bias_experts

    in_maps = []
    for c in range(N_CORES):
        gs = slice(c * G_PER_CORE, (c + 1) * G_PER_CORE)
        cg = coefficients[gs]  # [4, 8]
        cb_np = np.broadcast_to(
            cg.reshape(1, -1), (128, G_PER_CORE * N_EXPERTS)
        ).copy()
        cx_np = np.empty((NW, G_PER_CORE), np.float32)
        cx_np[0] = 1.0
        cx_np[1:] = cg.T
        cd_np = np.zeros((128, NW, 128), np.float16)
        idx = np.arange(128)
        cd_np[idx, 0, idx] = 1.0
        for e in range(N_EXPERTS):
            cd_np[idx, 1 + e, idx] = np.float16(cg[0, e])
        xT_np = np.ascontiguousarray(
            x16[c * TOK_PER_CORE : (c + 1) * TOK_PER_CORE].T
        )
        in_maps.append(
            {
                "xT": xT_np,
                "wallA0": wallA0_np,
                "wallA1": wallA1_np,
                "wallB0": wallB0_np,
                "wallB1": wallB1_np,
                "cdiag": cd_np,
                "cb": cb_np,
                "cx": cx_np,
                "ball": ball_np,
            }
        )

    res = run_bass_kernel_spmd(nc, in_maps, core_ids=list(range(N_CORES)))
    out = np.empty((N_CORES * TOK_PER_CORE, OUT_F), np.float32)
    for c in range(N_CORES):
        out[c * TOK_PER_CORE : (c + 1) * TOK_PER_CORE] = (
            np.asarray(res.results[c]["outT"]).T.astype(np.float32)
        )
    return out
